# revision 34
# baseline (speedup 1.0000x reference)
"""Multi-head causal attention (B=4,T=2048,C=1024,H=16,D=64) on 8 TRN2 NeuronCores.

Sharding: no collectives. Core c handles batch b=c//2 and a causally-balanced
set of four 256-query chunks (half=c%2): half0 -> chunks [7,5,2,0], half1 ->
[6,4,3,1]. Every core runs the same program with padded per-slot key-tile
counts [16,12,8,4]; per-core differences (real counts / causal diagonals) are
expressed purely through per-core input data (multiplicative 0/1 mask tiles).
K/V projections are computed for the full sequence on both cores of a batch.

v4 design (vs 568us fp32r baseline):
  * bf16 operands everywhere, fp32 PSUM accumulation (rel err ~3e-3).
  * kT / v / q / attn SBUF-resident -- no DRAM scratch roundtrip.
  * Startup DMA split across the two HW DGE queues (sync + scalar) in
    need-order: step-0 work is gated by ~2.5MB, not 19MB.
  * Fine-grained software pipelining: inside each attention c_run the ACT
    engine (exp, 1147ns/u-step) outpaces the PE's own 854ns/u-step, so
    independent projection matmuls (Q/K/V/out-proj "quanta") are woven
    between u-steps to keep the in-order PE stream busy while exp runs.
    The chunk needing key blocks 0..s runs at step s, right behind the
    K/V projection of block s.
  * Causal/pad masks are multiplicative 0/1 bf16 tiles applied on DVE to
    the exp output (not in-place: HW DVE corrupts on out==in0 APs).
"""

import numpy as np
import ml_dtypes

import concourse.bass as bass
import concourse.tile as tile
from concourse import bacc, library_config, mybir
from concourse.bass_utils import run_bass_kernel_spmd

B, T, C = 4, 2048, 1024
H, D = 16, 64
P = 128            # key tile size
QC = 256           # query chunk size
NP = 8             # head pairs
PN = [16, 12, 8, 4]                    # padded per-slot key-tile counts
CHUNKS = [[7, 5, 2, 0], [6, 4, 3, 1]]  # chunk ids per half, slot order
F32 = mybir.dt.float32
BF16 = mybir.dt.bfloat16
EXP = mybir.ActivationFunctionType.Exp
SCALE = float(C) ** -0.5
BF = ml_dtypes.bfloat16
DEBUG_DUMPS = False
AVPACK = False     # experimental: row-group-packed AV matmuls
PUMP = 2           # filler quanta pulled per u-step


def build_kernel(nc: bass.Bass):
    xT = nc.dram_tensor("xT", [C, T], BF16, kind="ExternalInput").ap()
    xq = nc.dram_tensor("xq", [C, 4 * QC], BF16, kind="ExternalInput").ap()
    wq2 = nc.dram_tensor("wq2", [C, C], BF16, kind="ExternalInput").ap()
    wk2 = nc.dram_tensor("wk2", [C, C], BF16, kind="ExternalInput").ap()
    wv2 = nc.dram_tensor("wv2", [C, C], BF16, kind="ExternalInput").ap()
    wp = nc.dram_tensor("wp", [C, C], BF16, kind="ExternalInput").ap()
    bias_in = nc.dram_tensor("bias_bc", [P, C], F32, kind="ExternalInput").ap()
    masks = nc.dram_tensor("masks", [16, P, QC], BF16, kind="ExternalInput").ap()
    out = nc.dram_tensor("out", [4, QC, C], F32, kind="ExternalOutput").ap()

    with tile.TileContext(nc) as tc:
        nc.gpsimd.load_library(library_config.attn)
        with (
            tc.tile_pool(name="const", bufs=1) as cpool,
            tc.tile_pool(name="xqp", bufs=8) as xqpool,
            tc.tile_pool(name="xt", bufs=8) as xtpool,
            tc.tile_pool(name="exp", bufs=2) as epool,
            tc.tile_pool(name="norm", bufs=1) as npool,
            tc.tile_pool(name="outp", bufs=1) as outpool,
            tc.tile_pool(name="ps_mm", bufs=2, space="PSUM") as ps_mm,
            tc.tile_pool(name="ps_sc", bufs=2, space="PSUM") as ps_sc,
            tc.tile_pool(name="ps_av", bufs=2, space="PSUM") as ps_av,
        ):
            qT_sb = cpool.tile([P, NP * 1024], BF16)       # Q^T, all 4 slots
            kT_sb = cpool.tile([P, NP * T], BF16)          # K^T, per head pair
            v_sb = cpool.tile([P, 16 * H * 65], BF16)      # V (+ones), per tile
            attn_sb = cpool.tile([P, 2 * NP * QC], BF16)   # 2-chunk rotation
            masks_sb = cpool.tile([P, 16 * QC], BF16)
            wq_sb = cpool.tile([P, 8 * 1024], BF16)
            wk_sb = cpool.tile([P, 8 * 1024], BF16)
            wv_sb = cpool.tile([P, 8 * 1024], BF16)
            wp_sb = cpool.tile([P, 8 * 1024], BF16)
            bias_bc = cpool.tile([P, C], F32)

            # ---- startup DMA, split across the two HWDGE queues ----------
            def loadw(eng, w_sb, w_dram, gs0, gs1):
                for g in range(gs0, gs1):
                    eng.dma_start(w_sb[:, g * 1024:(g + 1) * 1024],
                                  w_dram[g * P:(g + 1) * P, :])

            def load_xq(eng, k):
                xqg = [
                    xqpool.tile([P, QC], BF16, tag=f"xq{k % 2}",
                                name=f"xq{k}_{g}")
                    for g in range(8)
                ]
                for g in range(8):
                    eng.dma_start(xqg[g][:],
                                  xq[g * P:(g + 1) * P, k * QC:(k + 1) * QC])
                return xqg

            xtgs = {}

            def load_xt(tb, eng=nc.sync):
                xtg = [
                    xtpool.tile([P, 512], BF16, tag=f"xt{tb % 2}",
                                name=f"xt{tb}_{g}")
                    for g in range(8)
                ]
                for g in range(8):
                    eng.dma_start(xtg[g][:],
                                  xT[g * P:(g + 1) * P,
                                     tb * 512:(tb + 1) * 512])
                xtgs[tb] = xtg

            def load_masks(k, eng=nc.sync):
                for l in range(4):
                    i = 4 * k + l
                    eng.dma_start(masks_sb[:, i * QC:(i + 1) * QC], masks[i])

            xqgs = [None] * 4
            # sync queue:  wq lo | xq3 | wk lo | xt0 | masks3,2 | wv lo
            # scalar queue: wq hi | xq2 | wk hi | wv hi | xq1 xq0 | bias wp
            loadw(nc.sync, wq_sb, wq2, 0, 4)
            loadw(nc.scalar, wq_sb, wq2, 4, 8)
            xqgs[3] = load_xq(nc.sync, 3)
            xqgs[2] = load_xq(nc.scalar, 2)
            loadw(nc.sync, wk_sb, wk2, 0, 4)
            loadw(nc.scalar, wk_sb, wk2, 4, 8)
            load_xt(0)
            load_masks(3)
            loadw(nc.sync, wv_sb, wv2, 0, 4)
            loadw(nc.scalar, wv_sb, wv2, 4, 8)
            load_masks(2)
            load_masks(1)
            load_masks(0)
            xqgs[1] = load_xq(nc.scalar, 1)
            xqgs[0] = load_xq(nc.scalar, 0)
            nc.scalar.dma_start(bias_bc[:], bias_in[:])
            loadw(nc.scalar, wp_sb, wp, 0, 8)
            for j in range(16):
                vv_j = v_sb[:, j * H * 65:(j + 1) * H * 65].rearrange(
                    "p (h e) -> p h e", e=65
                )
                nc.vector.memset(vv_j[:, :, 64:65], 1.0)

            # ---- PE work units as generators of single-MM quanta ---------
            def b_unit(k, p):
                qps = ps_mm.tile([P, QC], F32, tag="mm", name=f"q{k}_{p}")
                for g in range(8):
                    nc.tensor.matmul(
                        qps[:],
                        wq_sb[:, g * 1024 + (2 * p) * 64:][:, :128],
                        xqgs[k][g][:],
                        start=(g == 0), stop=(g == 7),
                    )
                    yield
                nc.vector.tensor_copy(
                    qT_sb[:, p * 1024 + k * QC:][:, :QC], qps[:]
                )
                yield

            def a_k_unit(tb, p):
                kps = ps_mm.tile([P, 512], F32, tag="mm", name=f"k{tb}_{p}")
                for g in range(8):
                    nc.tensor.matmul(
                        kps[:],
                        wk_sb[:, g * 1024 + (2 * p) * 64:][:, :128],
                        xtgs[tb][g][:],
                        start=(g == 0), stop=(g == 7),
                    )
                    yield
                nc.vector.tensor_copy(
                    kT_sb[:, p * T + tb * 512:][:, :512], kps[:]
                )
                yield

            def a_v_unit(tb, sti):
                j = tb * 4 + sti
                vv = v_sb[:, j * H * 65:(j + 1) * H * 65].rearrange(
                    "p (h e) -> p h e", e=65
                )
                for hc in range(2):
                    vps = ps_mm.tile([P, 512], F32, tag="mm",
                                     name=f"v{j}_{hc}")
                    for g in range(8):
                        nc.tensor.matmul(
                            vps[:],
                            xtgs[tb][g][:, sti * P:(sti + 1) * P],
                            wv_sb[:, g * 1024 + hc * 512:][:, :512],
                            start=(g == 0), stop=(g == 7),
                        )
                        yield
                    nc.vector.tensor_copy(
                        vv[:, hc * 8:(hc + 1) * 8, 0:64],
                        vps[:].rearrange("p (h d) -> p h d", d=64),
                    )
                    yield

            def proj_unit(s, k, tt, oc):
                base = (s % 2) * NP * QC
                pp = ps_mm.tile([P, 512], F32, tag="mm", name=f"p{k}_{tt}{oc}")
                for g in range(NP):
                    nc.tensor.matmul(
                        pp[:],
                        attn_sb[:, base + g * QC + tt * P:][:, :P],
                        wp_sb[:, g * 1024 + oc * 512:][:, :512],
                        start=(g == 0), stop=(g == 7),
                    )
                    yield
                ot = outpool.tile([P, 512], F32, tag="ot")
                nc.vector.tensor_add(
                    ot[:], pp[:], bias_bc[:, oc * 512:(oc + 1) * 512]
                )
                nc.sync.dma_start(
                    out[k, tt * P:(tt + 1) * P, oc * 512:(oc + 1) * 512],
                    ot[:],
                )
                yield

            def load_xt_unit(tb):
                load_xt(tb)
                yield

            # global filler stream
            fill_q = []

            def pump(n):
                took = 0
                while took < n and fill_q:
                    try:
                        next(fill_q[0])
                        took += 1
                    except StopIteration:
                        fill_q.pop(0)

            def drain():
                pump(1 << 30)

            def c_run(s, k, p):
                avp = ps_av.tile([65, 2 * QC], F32, tag="av", name=f"av{k}_{p}")
                qA = qT_sb[0:64, p * 1024 + k * QC:][:, :QC]
                qB = qT_sb[64:128, p * 1024 + k * QC:][:, :QC]
                hA, hB = 2 * p, 2 * p + 1
                pend = None

                def emit_av(pv):
                    e_t, j0 = pv
                    for jj, j in enumerate((j0, j0 + 1)):
                        v0 = v_sb[:, j * H * 65 + hA * 65:][:, :65]
                        v1 = v_sb[:, j * H * 65 + hB * 65:][:, :65]
                        first = (j == 0)
                        last = (j == PN[k] - 1)
                        if AVPACK:
                            eA = e_t[:, jj * QC:(jj + 1) * QC]
                            eB = e_t[:, (2 + jj) * QC:(3 + jj) * QC]
                            nc.tensor.matmul(avp[:, 0:QC], v0[0:64, :],
                                             eA[0:64, :], start=first,
                                             stop=False, tile_position=(0, 0))
                            nc.tensor.matmul(avp[:, 0:QC], v0[64:128, :],
                                             eA[64:128, :], start=False,
                                             stop=False, tile_position=(64, 0))
                            nc.tensor.matmul(avp[:, QC:2 * QC], v1[0:64, :],
                                             eB[0:64, :], start=False,
                                             stop=False, tile_position=(0, 0))
                            nc.tensor.matmul(avp[:, QC:2 * QC], v1[64:128, :],
                                             eB[64:128, :], start=False,
                                             stop=last, tile_position=(64, 0))
                        else:
                            nc.tensor.matmul(avp[:, 0:QC], v0,
                                             e_t[:, jj * QC:(jj + 1) * QC],
                                             start=first, stop=False)
                            nc.tensor.matmul(avp[:, QC:2 * QC], v1,
                                             e_t[:, (2 + jj) * QC:(3 + jj) * QC],
                                             start=False, stop=last)

                for u in range(PN[k] // 2):
                    j0 = 2 * u
                    masked = j0 >= PN[k] - 4
                    kt0 = kT_sb[:, p * T + j0 * P:][:, :P]
                    kt1 = kT_sb[:, p * T + (j0 + 1) * P:][:, :P]
                    sc = ps_sc.tile([P, 4 * QC], F32, tag="sc")
                    nc.tensor.matmul(sc[:, 0:QC], kt0[0:64, :], qA,
                                     start=True, stop=True,
                                     tile_position=(0, 0))
                    nc.tensor.matmul(sc[:, 2 * QC:3 * QC], kt0[64:128, :],
                                     qB, start=True, stop=True,
                                     tile_position=(64, 0))
                    nc.tensor.matmul(sc[:, QC:2 * QC], kt1[0:64, :], qA,
                                     start=True, stop=True,
                                     tile_position=(0, 0))
                    nc.tensor.matmul(sc[:, 3 * QC:4 * QC], kt1[64:128, :],
                                     qB, start=True, stop=True,
                                     tile_position=(64, 0))
                    e_t = epool.tile([P, 4 * QC], BF16, tag="exp")
                    nc.scalar.activation(e_t[:], sc[:], EXP, scale=SCALE)
                    if masked:
                        li = (k * 4 + (j0 - (PN[k] - 4))) * QC
                        mb = masks_sb[:, li:li + 2 * QC]
                        e_m = epool.tile([P, 4 * QC], BF16, tag="expm",
                                         bufs=2)
                        nc.vector.tensor_mul(e_m[:, 0:2 * QC],
                                             e_t[:, 0:2 * QC], mb)
                        nc.vector.tensor_mul(e_m[:, 2 * QC:4 * QC],
                                             e_t[:, 2 * QC:4 * QC], mb)
                        e_t = e_m
                    if pend is not None:
                        emit_av(pend)
                    pend = (e_t, j0)
                    pump(PUMP)
                emit_av(pend)
                pump(1)

                rs = npool.tile([1, 2 * QC], F32, tag="rs")
                nc.vector.tensor_copy(rs[:], avp[64:65, :])
                rc = npool.tile([1, 2 * QC], F32, tag="rc")
                nc.vector.reciprocal_approx_fast(rc[:], rs[:])
                rb = npool.tile([64, 2 * QC], F32, tag="rb", bufs=2)
                nc.gpsimd.partition_broadcast(rb[:], rc[:])
                col = (s % 2) * NP * QC + p * QC
                nc.vector.tensor_mul(attn_sb[0:64, col:col + QC],
                                     avp[0:64, 0:QC], rb[:, 0:QC])
                nc.vector.tensor_mul(attn_sb[64:128, col:col + QC],
                                     avp[0:64, QC:2 * QC], rb[:, QC:2 * QC])

            # ---- emission schedule --------------------------------------
            def run_units(units):
                for u in units:
                    for _ in u:
                        pass

            # Prefix (gated by startup DMA): everything the first c_run needs.
            run_units([b_unit(3, p) for p in range(NP)])
            run_units([a_k_unit(0, p) for p in range(NP)])
            run_units([b_unit(2, p) for p in range(NP)])
            run_units([a_v_unit(0, j) for j in range(4)])
            load_xt(1)

            # phase 0: chunk slot 3 (PN=4); fillers: B1, A1, xt2 prefetch.
            fill_q.append(load_xt_unit(2))
            fill_q.extend(b_unit(1, p) for p in range(NP))
            fill_q.extend(a_k_unit(1, p) for p in range(NP))
            fill_q.extend(a_v_unit(1, j) for j in range(4))
            for p in range(NP):
                c_run(0, 3, p)
            drain()

            # phase 1: chunk slot 2 (PN=8); fillers: proj(slot3), A2, B0.
            fill_q.append(load_xt_unit(3))
            fill_q.extend(proj_unit(0, 3, tt, oc)
                          for tt in range(2) for oc in range(2))
            fill_q.extend(a_k_unit(2, p) for p in range(NP))
            fill_q.extend(a_v_unit(2, j) for j in range(4))
            fill_q.extend(b_unit(0, p) for p in range(NP))
            for p in range(NP):
                c_run(1, 2, p)
            drain()

            # merged phase 2+3: slot 1 (PN=12) then slot 0 (PN=16). Slot 0's
            # attention is ACT-bound with no K/V work left, so slot 1's
            # projection is deferred into it as PE filler.
            fill_q.extend(proj_unit(1, 2, tt, oc)
                          for tt in range(2) for oc in range(2))
            fill_q.extend(a_k_unit(3, p) for p in range(NP))
            fill_q.extend(a_v_unit(3, j) for j in range(4))
            for p in range(NP):
                c_run(2, 1, p)
            drain()
            fill_q.extend(proj_unit(2, 1, tt, oc)
                          for tt in range(2) for oc in range(2))
            for p in range(NP):
                c_run(3, 0, p)
            drain()
            run_units([proj_unit(3, 0, tt, oc)
                       for tt in range(2) for oc in range(2)])
    return nc


def _make_masks(half):
    """Multiplicative 0/1 masks for the 4 maybe-masked slots of each chunk."""
    chunks = CHUNKS[half]
    m = np.zeros((16, P, QC), np.float32)
    s = np.arange(P)[:, None]
    t = np.arange(QC)[None, :]
    for k in range(4):
        q = chunks[k]
        n = 2 * (q + 1)
        for l in range(4):
            j = PN[k] - 4 + l
            if j >= n:
                pat = np.zeros((P, QC), np.float32)
            elif j == n - 2:
                pat = (s <= t).astype(np.float32)
            elif j == n - 1:
                pat = (s <= t - 128).astype(np.float32)
            else:
                pat = np.ones((P, QC), np.float32)
            m[k * 4 + l] = pat
    return m.astype(BF)


_CACHE = {}


def _get_nc():
    if "nc" not in _CACHE:
        last = None
        for _ in range(4):
            try:
                nc = bacc.Bacc("TRN2", target_bir_lowering=False, debug=False)
                build_kernel(nc)
                nc.compile()
                _CACHE["nc"] = nc
                break
            except Exception as e:  # flaky scheduler race-check false positive
                if type(e).__name__ != "RaceCondition":
                    raise
                last = e
        else:
            raise last
    return _CACHE["nc"]


def make_in_maps(x, wq, wk, wv, w_proj, b_proj):
    x = np.asarray(x, np.float32)
    wq2 = np.ascontiguousarray(
        np.transpose(np.asarray(wq), (1, 0, 2)).reshape(C, C)).astype(BF)
    wk2 = np.ascontiguousarray(
        np.transpose(np.asarray(wk), (1, 0, 2)).reshape(C, C)).astype(BF)
    wv2 = np.ascontiguousarray(
        np.transpose(np.asarray(wv), (1, 0, 2)).reshape(C, C)).astype(BF)
    wpm = np.asarray(w_proj, np.float32).astype(BF)
    bias_bc = np.broadcast_to(
        np.asarray(b_proj, np.float32).reshape(1, C), (P, C)
    ).copy()
    masks_h = [_make_masks(0), _make_masks(1)]

    in_maps = []
    for core in range(8):
        b, half = core // 2, core % 2
        xTb = np.ascontiguousarray(x[b].T).astype(BF)
        xqb = np.ascontiguousarray(
            np.concatenate(
                [xTb[:, q * QC:(q + 1) * QC] for q in CHUNKS[half]], axis=1
            )
        )
        in_maps.append({
            "xT": xTb, "xq": xqb,
            "wq2": wq2, "wk2": wk2, "wv2": wv2,
            "wp": wpm, "bias_bc": bias_bc, "masks": masks_h[half],
        })
    return in_maps


def assemble(results):
    full = np.zeros((B, T, C), np.float32)
    for core in range(8):
        b, half = core // 2, core % 2
        o = results[core]["out"]
        for k, q in enumerate(CHUNKS[half]):
            full[b, q * QC:(q + 1) * QC] = o[k]
    return full


def kernel(x, wq, wk, wv, w_proj, b_proj, _trace=False, _tmpdir=None):
    in_maps = make_in_maps(x, wq, wk, wv, w_proj, b_proj)
    nc = _get_nc()
    res = run_bass_kernel_spmd(
        nc, in_maps, core_ids=list(range(8)), trace=_trace, tmpdir=_tmpdir
    )
    if _trace:
        _CACHE["last_result"] = res
    return assemble(res.results)


# revision 53
# speedup vs baseline: 1.0605x; 1.0605x over previous
"""Multi-head causal attention (B=4,T=2048,C=1024,H=16,D=64) on 8 TRN2 NeuronCores.

Sharding: no collectives. Core c handles batch b=c//2 and a causally-balanced
set of four 256-query chunks (half=c%2): half0 -> chunks [7,5,2,0], half1 ->
[6,4,3,1]. Every core runs the same program with padded per-slot key-tile
counts [16,12,8,4]; per-core differences (real counts / causal diagonals) are
expressed purely through per-core input data (multiplicative 0/1 mask tiles).
K/V projections are computed for the full sequence on both cores of a batch.

v4 design (vs 568us fp32r baseline):
  * bf16 operands everywhere, fp32 PSUM accumulation (rel err ~3e-3).
  * kT / v / q / attn SBUF-resident -- no DRAM scratch roundtrip.
  * Startup DMA split across the two HW DGE queues (sync + scalar) in
    need-order: step-0 work is gated by ~2.5MB, not 19MB.
  * Fine-grained software pipelining: inside each attention c_run the ACT
    engine (exp, 1147ns/u-step) outpaces the PE's own 854ns/u-step, so
    independent projection matmuls (Q/K/V/out-proj "quanta") are woven
    between u-steps to keep the in-order PE stream busy while exp runs.
    The chunk needing key blocks 0..s runs at step s, right behind the
    K/V projection of block s.
  * Causal/pad masks are multiplicative 0/1 bf16 tiles applied on DVE to
    the exp output (not in-place: HW DVE corrupts on out==in0 APs).
"""

import numpy as np
import ml_dtypes

import concourse.bass as bass
import concourse.tile as tile
from concourse import bacc, library_config, mybir
from concourse.bass_utils import run_bass_kernel_spmd

B, T, C = 4, 2048, 1024
H, D = 16, 64
P = 128            # key tile size
QC = 256           # query chunk size
NP = 8             # head pairs
PN = [16, 12, 8, 4]                    # padded per-slot key-tile counts
CHUNKS = [[7, 5, 2, 0], [6, 4, 3, 1]]  # chunk ids per half, slot order
F32 = mybir.dt.float32
BF16 = mybir.dt.bfloat16
F8 = mybir.dt.float8e4
DR = mybir.MatmulPerfMode.DoubleRow
EXP = mybir.ActivationFunctionType.Exp
SCALE = float(C) ** -0.5
BF = ml_dtypes.bfloat16
F8NP = ml_dtypes.float8_e4m3
DEBUG_DUMPS = False
PUMP = 2           # filler quanta pulled per u-step


def build_kernel(nc: bass.Bass):
    xT = nc.dram_tensor("xT", [C, T], BF16, kind="ExternalInput").ap()
    xq = nc.dram_tensor("xq", [C, 4 * QC], BF16, kind="ExternalInput").ap()
    wq2 = nc.dram_tensor("wq2", [C, C], BF16, kind="ExternalInput").ap()
    wk2 = nc.dram_tensor("wk2", [C, C], BF16, kind="ExternalInput").ap()
    wv2 = nc.dram_tensor("wv2", [C, C], BF16, kind="ExternalInput").ap()
    wp = nc.dram_tensor("wp", [C, C], BF16, kind="ExternalInput").ap()
    bias_in = nc.dram_tensor("bias_bc", [P, C], F32, kind="ExternalInput").ap()
    masks = nc.dram_tensor("masks", [16, P, QC], F8, kind="ExternalInput").ap()
    out = nc.dram_tensor("out", [4, QC, C], F32, kind="ExternalOutput").ap()

    with tile.TileContext(nc) as tc:
        nc.gpsimd.load_library(library_config.attn)
        with (
            tc.tile_pool(name="const", bufs=1) as cpool,
            tc.tile_pool(name="xqp", bufs=8) as xqpool,
            tc.tile_pool(name="xt", bufs=8) as xtpool,
            tc.tile_pool(name="exp", bufs=2) as epool,
            tc.tile_pool(name="norm", bufs=1) as npool,
            tc.tile_pool(name="outp", bufs=1) as outpool,
            tc.tile_pool(name="ps_mm", bufs=2, space="PSUM") as ps_mm,
            tc.tile_pool(name="ps_sc", bufs=2, space="PSUM") as ps_sc,
            tc.tile_pool(name="ps_av", bufs=2, space="PSUM") as ps_av,
        ):
            qT_sb = cpool.tile([P, NP * 1024], BF16)       # Q^T, all 4 slots
            kT_sb = cpool.tile([P, NP * T], BF16)          # K^T, per head pair
            v_sb = cpool.tile([P, 16 * H * 65], F8)        # V (+ones), per tile
            # bf16 copy of the first 4 v tiles: the short-context chunk
            # (slot 3) has a peaked softmax where fp8 weights fail tolerance.
            vb_sb = cpool.tile([P, 4 * H * 65], BF16)
            attn_sb = cpool.tile([P, 2 * NP * QC], BF16)   # 2-chunk rotation
            masks_sb = cpool.tile([P, 16 * QC], F8)
            masks3b = cpool.tile([P, 4 * QC], BF16)        # slot-3 masks, bf16
            wq_sb = cpool.tile([P, 8 * 1024], BF16)
            wk_sb = cpool.tile([P, 8 * 1024], BF16)
            wv_sb = cpool.tile([P, 8 * 1024], BF16)
            wp_sb = cpool.tile([P, 8 * 1024], BF16)
            bias_bc = cpool.tile([P, C], F32)

            # ---- startup DMA, split across the two HWDGE queues ----------
            def loadw(eng, w_sb, w_dram, gs0, gs1):
                for g in range(gs0, gs1):
                    eng.dma_start(w_sb[:, g * 1024:(g + 1) * 1024],
                                  w_dram[g * P:(g + 1) * P, :])

            def load_xq(eng, pair):
                """Load both chunk slots of a pair: slots {2*pair, 2*pair+1}."""
                xqg = [
                    xqpool.tile([P, 512], BF16, tag=f"xq{pair}",
                                name=f"xq{pair}_{g}")
                    for g in range(8)
                ]
                for g in range(8):
                    eng.dma_start(
                        xqg[g][:],
                        xq[g * P:(g + 1) * P, pair * 512:(pair + 1) * 512],
                    )
                return xqg

            xtgs = {}

            def load_xt(tb, eng=nc.sync):
                xtg = [
                    xtpool.tile([P, 512], BF16, tag=f"xt{tb % 2}",
                                name=f"xt{tb}_{g}")
                    for g in range(8)
                ]
                for g in range(8):
                    eng.dma_start(xtg[g][:],
                                  xT[g * P:(g + 1) * P,
                                     tb * 512:(tb + 1) * 512])
                xtgs[tb] = xtg

            def load_masks(k, eng=nc.sync):
                for l in range(4):
                    i = 4 * k + l
                    eng.dma_start(masks_sb[:, i * QC:(i + 1) * QC], masks[i])

            xqgs = [None] * 2
            # sync queue:  wq lo | xq pair1 lo | wk lo | xt0 | masks | wv lo
            # scalar queue: wq hi | xq pair1 hi | wk hi | wv hi | xq pair0 ...
            loadw(nc.sync, wq_sb, wq2, 0, 4)
            loadw(nc.scalar, wq_sb, wq2, 4, 8)
            xqgs[1] = load_xq(nc.sync, 1)
            loadw(nc.sync, wk_sb, wk2, 0, 4)
            loadw(nc.scalar, wk_sb, wk2, 4, 8)
            load_xt(0)
            load_masks(3)
            loadw(nc.sync, wv_sb, wv2, 0, 4)
            loadw(nc.scalar, wv_sb, wv2, 4, 8)
            load_masks(2)
            load_masks(1)
            load_masks(0)
            xqgs[0] = load_xq(nc.scalar, 0)
            nc.scalar.dma_start(bias_bc[:], bias_in[:])
            loadw(nc.scalar, wp_sb, wp, 0, 8)
            for j in range(16):
                vv_j = v_sb[:, j * H * 65:(j + 1) * H * 65].rearrange(
                    "p (h e) -> p h e", e=65
                )
                nc.vector.memset(vv_j[:, :, 64:65], 1.0)
            for j in range(4):
                vb_j = vb_sb[:, j * H * 65:(j + 1) * H * 65].rearrange(
                    "p (h e) -> p h e", e=65
                )
                nc.vector.memset(vb_j[:, :, 64:65], 1.0)
            nc.vector.tensor_copy(masks3b[:], masks_sb[:, 12 * QC:16 * QC])

            # ---- PE work units as generators of single-MM quanta ---------
            def b_unit(pair, p):
                """Q-projection for chunk-slot pair {2*pair, 2*pair+1}."""
                qps = ps_mm.tile([P, 512], F32, tag="mm", name=f"q{pair}_{p}")
                for g in range(8):
                    nc.tensor.matmul(
                        qps[:],
                        wq_sb[:, g * 1024 + (2 * p) * 64:][:, :128],
                        xqgs[pair][g][:],
                        start=(g == 0), stop=(g == 7),
                    )
                    yield
                nc.vector.tensor_copy(
                    qT_sb[:, p * 1024 + pair * 512:][:, :512], qps[:]
                )
                yield

            def a_k_unit(tb, p):
                kps = ps_mm.tile([P, 512], F32, tag="mm", name=f"k{tb}_{p}")
                for g in range(8):
                    nc.tensor.matmul(
                        kps[:],
                        wk_sb[:, g * 1024 + (2 * p) * 64:][:, :128],
                        xtgs[tb][g][:],
                        start=(g == 0), stop=(g == 7),
                    )
                    yield
                nc.vector.tensor_copy(
                    kT_sb[:, p * T + tb * 512:][:, :512], kps[:]
                )
                yield

            def a_v_unit(tb, sti):
                j = tb * 4 + sti
                vv = v_sb[:, j * H * 65:(j + 1) * H * 65].rearrange(
                    "p (h e) -> p h e", e=65
                )
                for hc in range(2):
                    vps = ps_mm.tile([P, 512], F32, tag="mm",
                                     name=f"v{j}_{hc}")
                    for g in range(8):
                        nc.tensor.matmul(
                            vps[:],
                            xtgs[tb][g][:, sti * P:(sti + 1) * P],
                            wv_sb[:, g * 1024 + hc * 512:][:, :512],
                            start=(g == 0), stop=(g == 7),
                        )
                        yield
                    nc.vector.tensor_copy(
                        vv[:, hc * 8:(hc + 1) * 8, 0:64],
                        vps[:].rearrange("p (h d) -> p h d", d=64),
                    )
                    if tb == 0:
                        vbv = vb_sb[:, j * H * 65:(j + 1) * H * 65].rearrange(
                            "p (h e) -> p h e", e=65
                        )
                        nc.vector.tensor_copy(
                            vbv[:, hc * 8:(hc + 1) * 8, 0:64],
                            vps[:].rearrange("p (h d) -> p h d", d=64),
                        )
                    yield

            def proj_unit(s, k, tt, oc):
                base = (s % 2) * NP * QC
                pp = ps_mm.tile([P, 512], F32, tag="mm", name=f"p{k}_{tt}{oc}")
                for g in range(NP):
                    nc.tensor.matmul(
                        pp[:],
                        attn_sb[:, base + g * QC + tt * P:][:, :P],
                        wp_sb[:, g * 1024 + oc * 512:][:, :512],
                        start=(g == 0), stop=(g == 7),
                    )
                    yield
                ot = outpool.tile([P, 512], F32, tag="ot")
                nc.vector.tensor_add(
                    ot[:], pp[:], bias_bc[:, oc * 512:(oc + 1) * 512]
                )
                nc.sync.dma_start(
                    out[k, tt * P:(tt + 1) * P, oc * 512:(oc + 1) * 512],
                    ot[:],
                )
                yield

            def load_xt_unit(tb):
                load_xt(tb)
                yield

            # global filler stream
            fill_q = []

            def pump(n):
                took = 0
                while took < n and fill_q:
                    try:
                        next(fill_q[0])
                        took += 1
                    except StopIteration:
                        fill_q.pop(0)

            def drain():
                pump(1 << 30)

            def c_run(s, k, p):
                hi = PN[k] == 4    # short-context chunk: bf16 AV path
                edt = BF16 if hi else F8
                avp = ps_av.tile([65, 2 * QC], F32, tag="av", name=f"av{k}_{p}")
                qA = qT_sb[0:64, p * 1024 + k * QC:][:, :QC]
                qB = qT_sb[64:128, p * 1024 + k * QC:][:, :QC]
                hA, hB = 2 * p, 2 * p + 1
                pend = None

                def emit_av(pv):
                    e_t, j0 = pv
                    first = (j0 == 0)
                    last = (j0 + 1 == PN[k] - 1)
                    if hi:
                        vb_all = vb_sb[:].rearrange("p (j h e) -> p j h e",
                                                    h=H, e=65)
                        for jj, j in enumerate((j0, j0 + 1)):
                            v0 = vb_all[:, j:j + 1, hA:hA + 1, 0:65]
                            v1 = vb_all[:, j:j + 1, hB:hB + 1, 0:65]
                            nc.tensor.matmul(avp[:, 0:QC], v0,
                                             e_t[:, jj * QC:(jj + 1) * QC],
                                             start=(j == 0), stop=False)
                            nc.tensor.matmul(avp[:, QC:2 * QC], v1,
                                             e_t[:, (2 + jj) * QC:(3 + jj) * QC],
                                             start=False,
                                             stop=(j == PN[k] - 1))
                        return
                    # fp8 DoubleRow: one matmul contracts BOTH key tiles of
                    # the pair (j0, j0+1). lhsT [128, 2, 65] strides to the
                    # two v tiles; rhs [128, 2, QC] to the two e_t blocks.
                    v_all = v_sb[:].rearrange("p (j h e) -> p j h e",
                                              h=H, e=65)
                    for hh, (h, reg) in enumerate(((hA, avp[:, 0:QC]),
                                                   (hB, avp[:, QC:2 * QC]))):
                        v2 = v_all[:, j0:j0 + 2, h:h + 1, 0:65]
                        e2 = e_t[:, 2 * hh * QC:(2 * hh + 2) * QC].rearrange(
                            "p (j q) -> p j q", j=2
                        )
                        nc.tensor.matmul(reg, v2, e2,
                                         start=(first and hh == 0),
                                         stop=(last and hh == 1),
                                         perf_mode=DR)

                for u in range(PN[k] // 2):
                    j0 = 2 * u
                    masked = j0 >= PN[k] - 4
                    kt0 = kT_sb[:, p * T + j0 * P:][:, :P]
                    kt1 = kT_sb[:, p * T + (j0 + 1) * P:][:, :P]
                    sc = ps_sc.tile([P, 4 * QC], F32, tag="sc")
                    nc.tensor.matmul(sc[:, 0:QC], kt0[0:64, :], qA,
                                     start=True, stop=True,
                                     tile_position=(0, 0))
                    nc.tensor.matmul(sc[:, 2 * QC:3 * QC], kt0[64:128, :],
                                     qB, start=True, stop=True,
                                     tile_position=(64, 0))
                    nc.tensor.matmul(sc[:, QC:2 * QC], kt1[0:64, :], qA,
                                     start=True, stop=True,
                                     tile_position=(0, 0))
                    nc.tensor.matmul(sc[:, 3 * QC:4 * QC], kt1[64:128, :],
                                     qB, start=True, stop=True,
                                     tile_position=(64, 0))
                    e_t = epool.tile([P, 4 * QC], edt, tag="exp", bufs=3)
                    nc.scalar.activation(e_t[:], sc[:], EXP, scale=SCALE)
                    if masked:
                        if hi:
                            mb = masks3b[:, (j0 - (PN[k] - 4)) * QC:][:, :2 * QC]
                        else:
                            li = (k * 4 + (j0 - (PN[k] - 4))) * QC
                            mb = masks_sb[:, li:li + 2 * QC]
                        e_m = epool.tile([P, 4 * QC], edt, tag="expm",
                                         bufs=2)
                        nc.vector.tensor_mul(e_m[:, 0:2 * QC],
                                             e_t[:, 0:2 * QC], mb)
                        nc.vector.tensor_mul(e_m[:, 2 * QC:4 * QC],
                                             e_t[:, 2 * QC:4 * QC], mb)
                        e_t = e_m
                    if pend is not None:
                        emit_av(pend)
                    pend = (e_t, j0)
                    pump(PUMP)
                emit_av(pend)
                pump(1)

                rs = npool.tile([1, 2 * QC], F32, tag="rs")
                nc.vector.tensor_copy(rs[:], avp[64:65, :])
                rc = npool.tile([1, 2 * QC], F32, tag="rc")
                nc.vector.reciprocal_approx_fast(rc[:], rs[:])
                rb = npool.tile([64, 2 * QC], F32, tag="rb", bufs=2)
                nc.gpsimd.partition_broadcast(rb[:], rc[:])
                col = (s % 2) * NP * QC + p * QC
                nc.vector.tensor_mul(attn_sb[0:64, col:col + QC],
                                     avp[0:64, 0:QC], rb[:, 0:QC])
                nc.vector.tensor_mul(attn_sb[64:128, col:col + QC],
                                     avp[0:64, QC:2 * QC], rb[:, QC:2 * QC])

            # ---- emission schedule --------------------------------------
            def run_units(units):
                for u in units:
                    for _ in u:
                        pass

            # Prefix (gated by startup DMA): everything the first c_run needs.
            run_units([b_unit(1, p) for p in range(NP)])   # Q for slots 3+2
            run_units([a_k_unit(0, p) for p in range(NP)])
            run_units([a_v_unit(0, j) for j in range(4)])
            load_xt(1)

            # phase 0: chunk slot 3 (PN=4); fillers: B pair0, A1, xt2.
            fill_q.append(load_xt_unit(2))
            fill_q.extend(b_unit(0, p) for p in range(NP))  # Q for slots 1+0
            fill_q.extend(a_k_unit(1, p) for p in range(NP))
            fill_q.extend(a_v_unit(1, j) for j in range(4))
            for p in range(NP):
                c_run(0, 3, p)
            drain()

            # phase 1: chunk slot 2 (PN=8); fillers: proj(slot3), A2.
            fill_q.append(load_xt_unit(3))
            fill_q.extend(proj_unit(0, 3, tt, oc)
                          for tt in range(2) for oc in range(2))
            fill_q.extend(a_k_unit(2, p) for p in range(NP))
            fill_q.extend(a_v_unit(2, j) for j in range(4))
            for p in range(NP):
                c_run(1, 2, p)
            drain()

            # merged phase 2+3: slot 1 (PN=12) then slot 0 (PN=16). Slot 0's
            # attention is ACT-bound with no K/V work left, so slot 1's
            # projection is deferred into it as PE filler.
            fill_q.extend(proj_unit(1, 2, tt, oc)
                          for tt in range(2) for oc in range(2))
            fill_q.extend(a_k_unit(3, p) for p in range(NP))
            fill_q.extend(a_v_unit(3, j) for j in range(4))
            for p in range(NP):
                c_run(2, 1, p)
            drain()
            fill_q.extend(proj_unit(2, 1, tt, oc)
                          for tt in range(2) for oc in range(2))
            for p in range(NP):
                c_run(3, 0, p)
            drain()
            run_units([proj_unit(3, 0, tt, oc)
                       for tt in range(2) for oc in range(2)])
    return nc


def _make_masks(half):
    """Multiplicative 0/1 masks for the 4 maybe-masked slots of each chunk."""
    chunks = CHUNKS[half]
    m = np.zeros((16, P, QC), np.float32)
    s = np.arange(P)[:, None]
    t = np.arange(QC)[None, :]
    for k in range(4):
        q = chunks[k]
        n = 2 * (q + 1)
        for l in range(4):
            j = PN[k] - 4 + l
            if j >= n:
                pat = np.zeros((P, QC), np.float32)
            elif j == n - 2:
                pat = (s <= t).astype(np.float32)
            elif j == n - 1:
                pat = (s <= t - 128).astype(np.float32)
            else:
                pat = np.ones((P, QC), np.float32)
            m[k * 4 + l] = pat
    return m.astype(F8NP)


_CACHE = {}


def _get_nc():
    if "nc" not in _CACHE:
        last = None
        for _ in range(4):
            try:
                nc = bacc.Bacc("TRN2", target_bir_lowering=False, debug=False)
                build_kernel(nc)
                nc.compile()
                _CACHE["nc"] = nc
                break
            except Exception as e:  # flaky scheduler race-check false positive
                if type(e).__name__ != "RaceCondition":
                    raise
                last = e
        else:
            raise last
    return _CACHE["nc"]


def make_in_maps(x, wq, wk, wv, w_proj, b_proj):
    x = np.asarray(x, np.float32)
    wq2 = np.ascontiguousarray(
        np.transpose(np.asarray(wq), (1, 0, 2)).reshape(C, C)).astype(BF)
    wk2 = np.ascontiguousarray(
        np.transpose(np.asarray(wk), (1, 0, 2)).reshape(C, C)).astype(BF)
    wv2 = np.ascontiguousarray(
        np.transpose(np.asarray(wv), (1, 0, 2)).reshape(C, C)).astype(BF)
    wpm = np.asarray(w_proj, np.float32).astype(BF)
    bias_bc = np.broadcast_to(
        np.asarray(b_proj, np.float32).reshape(1, C), (P, C)
    ).copy()
    masks_h = [_make_masks(0), _make_masks(1)]

    in_maps = []
    for core in range(8):
        b, half = core // 2, core % 2
        xTb = np.ascontiguousarray(x[b].T).astype(BF)
        xqb = np.ascontiguousarray(
            np.concatenate(
                [xTb[:, q * QC:(q + 1) * QC] for q in CHUNKS[half]], axis=1
            )
        )
        in_maps.append({
            "xT": xTb, "xq": xqb,
            "wq2": wq2, "wk2": wk2, "wv2": wv2,
            "wp": wpm, "bias_bc": bias_bc, "masks": masks_h[half],
        })
    return in_maps


def assemble(results):
    full = np.zeros((B, T, C), np.float32)
    for core in range(8):
        b, half = core // 2, core % 2
        o = results[core]["out"]
        for k, q in enumerate(CHUNKS[half]):
            full[b, q * QC:(q + 1) * QC] = o[k]
    return full


def kernel(x, wq, wk, wv, w_proj, b_proj, _trace=False, _tmpdir=None):
    in_maps = make_in_maps(x, wq, wk, wv, w_proj, b_proj)
    nc = _get_nc()
    res = run_bass_kernel_spmd(
        nc, in_maps, core_ids=list(range(8)), trace=_trace, tmpdir=_tmpdir
    )
    if _trace:
        _CACHE["last_result"] = res
    return assemble(res.results)
